# revision 28
# baseline (speedup 1.0000x reference)
"""Trainium2 Bass kernel for causal multi-head attention (v3, fused pipeline).

Problem: x[1,4096,1024] -> MHA(16 heads, head_dim 64, causal) -> out[1,4096,1024]
  q,k,v = x @ W_{q,k,v}; scores = q k^T / 8 (causal); out = softmax(scores) v @ W_o + b_o

Sharding: tensor-parallel over heads, 2 heads (128 feature dims) per core.
Each core computes a full-width partial output ctx_c @ W_o[slice_c] in bf16;
the host sums the 8 partials (row-parallel out-projection).

Design notes:
  * Single fused software-pipelined loop: QKV projection for chunk c+1,
    the normalization epilogue of chunk c-1 and its out-projection are all
    interleaved into the (ACT-exp-bound) attention steps of chunk c, keeping
    the PE dense and HAM-warm.  PV matmuls trail the score matmuls by one
    step so the scalar engine's exp stream never stalls on the PE FIFO at
    chunk boundaries.
  * Diagonal blocks are sliced: score/exp/PV only touch the causally-valid
    q-range [128j, 512); exp handles both heads' valid slices in ONE
    ACTIVATE via a [128, 2, L] access pattern.
  * Softmax normalization is applied to ctx^T before the out-projection
    (per-q reciprocal broadcast to all partitions with a K=1 bf16 matmul +
    reciprocal_approx_fast), so the out-projection collapses to ONE K=128
    matmul per output tile.
  * PSUM: sm 2x2 banks, ctx 2 banks, generic rotating 2 banks = 8.
  * Output partials are written as bf16 (halves DMA).

Numerics: softmax without max-subtraction (|scores/8| < ~3 here); bf16
matmul inputs; fp32 PSUM accumulation; bf16 softmax denominators ->
~5e-3 max-rel vs fp32 reference (tolerance 2e-2).

kernel(**inputs) takes the FULL unsharded inputs and returns the FULL output.
"""

import sys

import numpy as np

for _p in ("/opt/trn_rl_repo", "/root/.axon_site/_ro/trn_rl_repo"):
    if _p not in sys.path:
        try:
            import concourse  # noqa: F401

            break
        except ImportError:
            sys.path.insert(0, _p)

N_CORES = 8
SEQ = 4096
D = 1024
DC = 128  # per-core slice of the head dim (2 heads x 64)
HD = 64


def build_bass(n=SEQ, d=D):
    """Trace the per-core SPMD Bass program. n = sequence length."""
    import concourse.bacc as bacc
    import concourse.mybir as mybir
    import concourse.tile as tile
    from concourse.masks import make_identity

    fp32 = mybir.dt.float32
    bf16 = mybir.dt.bfloat16
    Exp = mybir.ActivationFunctionType.Exp

    assert n % 512 == 0 and d % 128 == 0
    NCH = n // 512  # 512-row q chunks
    NT = n // 128  # 128-row seq tiles
    DIT = d // 128  # input-dim 128-tiles
    SCALE = 1.0 / float(np.sqrt(HD))

    nc = bacc.Bacc("TRN2", target_bir_lowering=False)

    xT_d = nc.dram_tensor("xT", (d, n), bf16, kind="ExternalInput")
    wq_d = nc.dram_tensor("wq", (d, DC), bf16, kind="ExternalInput")
    wk_d = nc.dram_tensor("wk", (d, DC), bf16, kind="ExternalInput")
    wv_d = nc.dram_tensor("wv", (d, DC), bf16, kind="ExternalInput")
    wo_d = nc.dram_tensor("wo", (DC, d), bf16, kind="ExternalInput")
    out_d = nc.dram_tensor("out", (n, d), bf16, kind="ExternalOutput")

    with tile.TileContext(nc) as tc:
        with (
            tc.tile_pool(name="const", bufs=1) as const_pool,
            tc.tile_pool(name="weights", bufs=1) as w_pool,
            tc.tile_pool(name="big", bufs=1) as big_pool,
            tc.tile_pool(name="xin", bufs=2) as xin_pool,
            tc.tile_pool(name="vt", bufs=2) as vt_pool,
            tc.tile_pool(name="pw", bufs=4) as p_pool,
            tc.tile_pool(name="ep", bufs=2) as ep_pool,
            tc.tile_pool(name="outsb", bufs=3) as out_pool,
            tc.tile_pool(name="ps", bufs=1, space="PSUM") as ps,
        ):
            # ---- input DMAs first: q weights + x chunk 0 gate the pipeline
            wq_sb = w_pool.tile([128, DIT, DC], bf16)
            nc.sync.dma_start(wq_sb[:], wq_d[:].rearrange("(t p) c -> p t c", p=128))
            xch0 = xin_pool.tile([128, DIT, 512], bf16, tag="xch", bufs=3)
            nc.sync.dma_start(
                xch0[:, 0 : DIT // 2, :],
                xT_d[0 : d // 2, 0:512].rearrange("(t p) c -> p t c", p=128),
            )
            nc.sync.dma_start(
                xch0[:, DIT // 2 :, :],
                xT_d[d // 2 :, 0:512].rearrange("(t p) c -> p t c", p=128),
            )
            wk_sb = w_pool.tile([128, DIT, DC], bf16)
            wv_sb = w_pool.tile([128, DIT, DC], bf16)
            nc.sync.dma_start(wk_sb[:], wk_d[:].rearrange("(t p) c -> p t c", p=128))
            nc.sync.dma_start(wv_sb[:], wv_d[:].rearrange("(t p) c -> p t c", p=128))
            wo_sb = w_pool.tile([DC, d], bf16)
            nc.sync.dma_start(wo_sb[:], wo_d[:])

            # ---- constants ----
            ident = const_pool.tile([128, 128], bf16)
            make_identity(nc, ident[:])
            # PE warm-up burst: ~3.5us of real matmul activity releases the
            # HAM clock throttle before the first projection arrives.
            warm_ps = ps.tile([128, 128], fp32, tag="gen", bufs=2,
                              padded_shape=[128, 512], name="gen_w")
            for _ in range(36):
                nc.tensor.matmul(warm_ps[:], ident[:], ident[:],
                                 start=True, stop=True)
            warm_sink = const_pool.tile([1, 128], fp32)
            nc.vector.tensor_copy(warm_sink[:], warm_ps[0:1, :])
            ones_row = const_pool.tile([1, 128], bf16)
            nc.gpsimd.memset(ones_row[:], 1.0)
            # Diagonal causal masks: mask[j][kl, ql] = 1 if ql >= kl + 128*j
            masks = const_pool.tile([128, 4, 512], bf16)
            nc.gpsimd.memset(masks[:], 1.0)
            for dd in range(4):
                nc.gpsimd.affine_select(
                    out=masks[:, dd, :],
                    in_=masks[:, dd, :],
                    compare_op=mybir.AluOpType.is_ge,
                    fill=0.0,
                    base=-128 * dd,
                    pattern=[[1, 512]],
                    channel_multiplier=-1,
                )

            # ---- persistent activations ----
            qt_sb = big_pool.tile([DC, n], bf16)  # Q^T (head dims on partitions)
            kt_sb = big_pool.tile([DC, n], bf16)  # K^T
            # V natural, augmented with ones columns at 64 (h0) and 144 (h1).
            # Per-head slices start at 32B-aligned offsets (0 and 160B) so the
            # DMA-xbar transposes can write them directly.
            v_aug = big_pool.tile([128, NT, 160], bf16)
            onescol = const_pool.tile([128, NT], fp32)
            nc.gpsimd.memset(onescol[:], 1.0)
            nc.vector.tensor_copy(v_aug[:, :, HD], onescol[:])
            nc.vector.tensor_copy(v_aug[:, :, 80 + HD], onescol[:])
            ctxn_sb = big_pool.tile([DC, n], bf16)  # normalized ctx^T

            # ---------- emission helpers ----------
            bg_queue = []  # deferred background closures (PE-filler work)

            def drain_bg(k):
                for _ in range(k):
                    if not bg_queue:
                        return
                    bg_queue.pop(0)()

            def emit_xch_dma(c):
                c0, c1 = c * 512, c * 512 + 512
                xch = xin_pool.tile([128, DIT, 512], bf16, tag="xch", bufs=3)
                nc.sync.dma_start(
                    xch[:], xT_d[:, c0:c1].rearrange("(t p) c -> p t c", p=128)
                )
                return xch

            def proj_units(c, xch, w_sb, evict):
                """Two 4-matmul halves of one projection (finer PE granularity)."""
                state = {}

                def first():
                    state["acc"] = ps.tile(
                        [128, 512], fp32, tag="gen", bufs=2, name="gen"
                    )
                    for dit in range(DIT // 2):
                        nc.tensor.matmul(
                            state["acc"][:], w_sb[:, dit, :], xch[:, dit, :],
                            start=(dit == 0), stop=False,
                        )

                def second():
                    for dit in range(DIT // 2, DIT):
                        nc.tensor.matmul(
                            state["acc"][:], w_sb[:, dit, :], xch[:, dit, :],
                            start=False, stop=(dit == DIT - 1),
                        )
                    evict(state["acc"])

                return [first, second]

            def qk_units(c, xch):
                qs = slice(c * 512, c * 512 + 512)

                def q_evict(acc):
                    nc.vector.tensor_copy(qt_sb[:, qs], acc[:])

                def k_evict(acc):
                    nc.vector.tensor_copy(kt_sb[:, qs], acc[:])

                return (proj_units(c, xch, wq_sb, q_evict)
                        + proj_units(c, xch, wk_sb, k_evict))

            def v_units(c, xch):
                vt_t = vt_pool.tile([DC, 512], bf16, tag="vt", bufs=3)

                def v_evict(acc):
                    nc.vector.tensor_copy(vt_t[:], acc[:])

                units = proj_units(c, xch, wv_sb, v_evict)

                def vtrans(j):
                    def unit():
                        ti = c * 4 + j
                        tp = ps.tile(
                            [128, 128], bf16, tag="gen", bufs=2,
                            padded_shape=[128, 512], name="gen_tp",
                        )
                        nc.tensor.transpose(
                            tp[:], vt_t[:, j * 128 : (j + 1) * 128], ident[:]
                        )
                        # one strided copy fills h0 cols 0:64 and h1 cols 80:144
                        nc.vector.tensor_copy(
                            v_aug[:, ti, 0:160].rearrange("p (g c) -> p g c", g=2)[
                                :, :, 0:HD
                            ],
                            tp[:].rearrange("p (g c) -> p g c", g=2),
                        )
                    return unit

                units += [vtrans(j) for j in range(4)]
                return units

            def score_exp(c, kt, sm_out):
                """Scores + exp for one step; returns the pm tile."""
                j = kt - 4 * c  # >= 0 on diagonal blocks
                qlo = 128 * j if j >= 0 else 0
                qs0 = c * 512
                kc = slice(kt * 128, kt * 128 + 128)
                nc.tensor.matmul(
                    sm_out[:, 0, qlo:512], kt_sb[0:HD, kc],
                    qt_sb[0:HD, qs0 + qlo : qs0 + 512],
                    start=True, stop=True, tile_position=(0, 0),
                )
                nc.tensor.matmul(
                    sm_out[:, 1, qlo:512], kt_sb[HD:DC, kc],
                    qt_sb[HD:DC, qs0 + qlo : qs0 + 512],
                    start=True, stop=True, tile_position=(64, 0),
                )
                pm = p_pool.tile([128, 2, 512], bf16, tag="p", bufs=4)
                nc.scalar.activation(
                    pm[:, :, qlo:512], sm_out[:, :, qlo:512], Exp, scale=SCALE
                )
                if j >= 0:
                    nc.vector.tensor_mul(
                        pm[:, 0, qlo:512], pm[:, 0, qlo:512], masks[:, j, qlo:512]
                    )
                    nc.vector.tensor_mul(
                        pm[:, 1, qlo:512], pm[:, 1, qlo:512], masks[:, j, qlo:512]
                    )
                return pm

            def pv_step(c, kt, nkt, ctx, pm):
                j = kt - 4 * c
                qlo = 128 * j if j >= 0 else 0
                nc.tensor.matmul(
                    ctx[:, 0, qlo:512], v_aug[:, kt, 0 : HD + 1], pm[:, 0, qlo:512],
                    start=(kt == 0), stop=(kt == nkt - 1),
                )
                nc.tensor.matmul(
                    ctx[:, 1, qlo:512], v_aug[:, kt, 80 : 80 + HD + 1],
                    pm[:, 1, qlo:512],
                    start=(kt == 0), stop=(kt == nkt - 1),
                )

            def epilogue_dve(c, ctx, last=False):
                """Immediately free the ctx PSUM banks: sums + raw eviction.

                The scalar engine is idle at chunk boundaries, so it takes the
                sums row and one ctx half in parallel with the DVE half —
                ctx's banks release ~2.5x sooner, unblocking the next chunk's
                first PV accumulation.
                """
                sums = ep_pool.tile([1, 2, 512], bf16, tag="sums", bufs=2)
                nc.vector.tensor_copy(sums[:], ctx[HD : HD + 1, :, :])
                craw = ep_pool.tile([128, 1024], bf16, tag="craw", bufs=2)
                nc.vector.tensor_copy(craw[0:HD, 0:512], ctx[0:HD, 0, :])
                nc.vector.tensor_copy(craw[HD:DC, 512:1024], ctx[0:HD, 1, :])
                return sums, craw

            def norm_units(c, sums, craw):
                """Normalize ctx^T for chunk c -> ctxn_sb (background units)."""
                qs = slice(c * 512, c * 512 + 512)
                rscb = ep_pool.tile([128, 1024], fp32, tag="rscb", bufs=2)

                def bcast(h):
                    def unit():
                        smb = ps.tile(
                            [128, 512], fp32, tag="gen", bufs=2, name="gen_b"
                        )
                        nc.tensor.matmul(
                            smb[:], ones_row[:], sums[:, h, :],
                            start=True, stop=True,
                        )
                        nc.vector.reciprocal_approx_fast(
                            rscb[:, h * 512 : h * 512 + 512], smb[:]
                        )
                    return unit

                def tnorm():
                    nc.vector.tensor_mul(
                        ctxn_sb[0:HD, qs], craw[0:HD, 0:512], rscb[0:HD, 0:512]
                    )
                    nc.vector.tensor_mul(
                        ctxn_sb[HD:DC, qs], craw[HD:DC, 512:1024],
                        rscb[HD:DC, 512:1024],
                    )

                return [bcast(0), bcast(1), tnorm]

            def outproj_units(c, last=False):
                """Out-projection for chunk c: 4 q-tiles x 2 d-halves."""
                units = []
                for jq in range(4):
                    jj = c * 4 + jq
                    gsl = slice(jj * 128, jj * 128 + 128)
                    o_sb = out_pool.tile([128, d], bf16, tag="o", bufs=3)

                    def half(h2, jj=jj, gsl=gsl, o_sb=o_sb):
                        def unit():
                            osl = slice(h2 * 512, h2 * 512 + 512)
                            op = ps.tile(
                                [128, 512], fp32, tag="gen", bufs=2, name="gen_o"
                            )
                            nc.tensor.matmul(
                                op[:], ctxn_sb[:, gsl], wo_sb[:, osl],
                                start=True, stop=True,
                            )
                            if last and h2 == 0:
                                nc.scalar.copy(o_sb[:, osl], op[:])
                            else:
                                nc.vector.tensor_copy(o_sb[:, osl], op[:])
                            if h2 == 1:
                                nc.sync.dma_start(out_d[gsl, :], o_sb[:])
                        return unit

                    units += [half(0), half(1)]
                return units

            # ---------- fused main loop ----------
            # prologue: chunk 0 q/k projections inline; v + transposes in bg.
            # QKV for chunk c+2 is enqueued during attention of chunk c
            # (pipeline depth 2) so the thin early chunks keep the PE dense.
            for u in qk_units(0, xch0):
                u()
            bg_queue.extend(v_units(0, xch0))
            xch1 = emit_xch_dma(1)
            bg_queue.extend(qk_units(1, xch1))
            bg_queue.extend(v_units(1, xch1))
            for c in range(NCH):
                if c + 2 < NCH:
                    xch = emit_xch_dma(c + 2)
                    bg_queue.extend(qk_units(c + 2, xch))
                    bg_queue.extend(v_units(c + 2, xch))
                nkt = 4 * (c + 1)
                ctx = ps.tile([HD + 1, 2, 512], fp32, tag="ctx", bufs=1, name="ctx")
                per_step = max(1, -(-len(bg_queue) // nkt))
                pm_prev = None
                for kt in range(nkt):
                    sm = ps.tile([128, 2, 512], fp32, tag="s", bufs=2, name="sm")
                    pm = score_exp(c, kt, sm)
                    if pm_prev is not None:
                        pv_step(c, kt - 1, nkt, ctx, pm_prev)
                    pm_prev = pm
                    drain_bg(per_step)
                pv_step(c, nkt - 1, nkt, ctx, pm_prev)
                drain_bg(len(bg_queue))
                last = c == NCH - 1
                sums, craw = epilogue_dve(c, ctx, last=last)
                bg_queue.extend(norm_units(c, sums, craw))
                bg_queue.extend(outproj_units(c, last=last))
            drain_bg(len(bg_queue))

    nc.compile()
    return nc


_NC_CACHE = {}


def _get_nc(n=SEQ):
    if n not in _NC_CACHE:
        _NC_CACHE[n] = build_bass(n)
    return _NC_CACHE[n]


def make_in_maps(x, W_q, W_k, W_v, W_o):
    import ml_dtypes

    bf16 = ml_dtypes.bfloat16
    n = x.shape[-2]
    xT = np.ascontiguousarray(
        np.asarray(x, dtype=np.float32).reshape(n, D).T
    ).astype(bf16)
    in_maps = []
    for c in range(N_CORES):
        s = slice(c * DC, (c + 1) * DC)
        in_maps.append(
            {
                "xT": xT,
                "wq": np.ascontiguousarray(np.asarray(W_q, np.float32)[:, s]).astype(bf16),
                "wk": np.ascontiguousarray(np.asarray(W_k, np.float32)[:, s]).astype(bf16),
                "wv": np.ascontiguousarray(np.asarray(W_v, np.float32)[:, s]).astype(bf16),
                "wo": np.ascontiguousarray(np.asarray(W_o, np.float32)[s, :]).astype(bf16),
            }
        )
    return in_maps


def kernel(x, W_q, W_k, W_v, W_o, b_o):
    from concourse import bass_utils

    x = np.asarray(x)
    b, n, _ = x.shape
    assert b == 1 and n == SEQ

    nc = _get_nc(n)
    in_maps = make_in_maps(x, W_q, W_k, W_v, W_o)
    res = bass_utils.run_bass_kernel_spmd(nc, in_maps, list(range(N_CORES)))
    acc = np.zeros((n, D), dtype=np.float64)
    for r in res.results:
        acc += np.asarray(r["out"], dtype=np.float64)
    acc += np.asarray(b_o, np.float64)[None, :]
    return acc.astype(np.float32).reshape(1, n, D)


# revision 29
# speedup vs baseline: 1.1950x; 1.1950x over previous
"""Trainium2 Bass kernel for causal multi-head attention (v3, fused pipeline).

Problem: x[1,4096,1024] -> MHA(16 heads, head_dim 64, causal) -> out[1,4096,1024]
  q,k,v = x @ W_{q,k,v}; scores = q k^T / 8 (causal); out = softmax(scores) v @ W_o + b_o

Sharding: tensor-parallel over heads, 2 heads (128 feature dims) per core.
Each core computes a full-width partial output ctx_c @ W_o[slice_c] in bf16;
the host sums the 8 partials (row-parallel out-projection).

Design notes:
  * Single fused software-pipelined loop: QKV projection for chunk c+1,
    the normalization epilogue of chunk c-1 and its out-projection are all
    interleaved into the (ACT-exp-bound) attention steps of chunk c, keeping
    the PE dense and HAM-warm.  PV matmuls trail the score matmuls by one
    step so the scalar engine's exp stream never stalls on the PE FIFO at
    chunk boundaries.
  * Diagonal blocks are sliced: score/exp/PV only touch the causally-valid
    q-range [128j, 512); exp handles both heads' valid slices in ONE
    ACTIVATE via a [128, 2, L] access pattern.
  * Softmax normalization is applied to ctx^T before the out-projection
    (per-q reciprocal broadcast to all partitions with a K=1 bf16 matmul +
    reciprocal_approx_fast), so the out-projection collapses to ONE K=128
    matmul per output tile.
  * PSUM: sm 2x2 banks, ctx 2 banks, generic rotating 2 banks = 8.
  * Output partials are written as bf16 (halves DMA).

Numerics: softmax without max-subtraction (|scores/8| < ~3 here); bf16
matmul inputs; fp32 PSUM accumulation; bf16 softmax denominators ->
~5e-3 max-rel vs fp32 reference (tolerance 2e-2).

kernel(**inputs) takes the FULL unsharded inputs and returns the FULL output.
"""

import sys

import numpy as np

for _p in ("/opt/trn_rl_repo", "/root/.axon_site/_ro/trn_rl_repo"):
    if _p not in sys.path:
        try:
            import concourse  # noqa: F401

            break
        except ImportError:
            sys.path.insert(0, _p)

N_CORES = 8
SEQ = 4096
D = 1024
DC = 128  # per-core slice of the head dim (2 heads x 64)
HD = 64


def build_bass(n=SEQ, d=D):
    """Trace the per-core SPMD Bass program. n = sequence length."""
    import concourse.bacc as bacc
    import concourse.mybir as mybir
    import concourse.tile as tile
    from concourse.masks import make_identity

    fp32 = mybir.dt.float32
    bf16 = mybir.dt.bfloat16
    Exp = mybir.ActivationFunctionType.Exp

    assert n % 512 == 0 and d % 128 == 0
    NCH = n // 512  # 512-row q chunks
    NT = n // 128  # 128-row seq tiles
    DIT = d // 128  # input-dim 128-tiles
    SCALE = 1.0 / float(np.sqrt(HD))

    nc = bacc.Bacc("TRN2", target_bir_lowering=False)

    xT_d = nc.dram_tensor("xT", (d, n), bf16, kind="ExternalInput")
    wq_d = nc.dram_tensor("wq", (d, DC), bf16, kind="ExternalInput")
    wk_d = nc.dram_tensor("wk", (d, DC), bf16, kind="ExternalInput")
    wv_d = nc.dram_tensor("wv", (d, DC), bf16, kind="ExternalInput")
    wo_d = nc.dram_tensor("wo", (DC, d), bf16, kind="ExternalInput")
    out_d = nc.dram_tensor("out", (n, d), bf16, kind="ExternalOutput")

    with tile.TileContext(nc) as tc:
        with (
            tc.tile_pool(name="const", bufs=1) as const_pool,
            tc.tile_pool(name="weights", bufs=1) as w_pool,
            tc.tile_pool(name="big", bufs=1) as big_pool,
            tc.tile_pool(name="xin", bufs=2) as xin_pool,
            tc.tile_pool(name="vt", bufs=2) as vt_pool,
            tc.tile_pool(name="pw", bufs=4) as p_pool,
            tc.tile_pool(name="ep", bufs=2) as ep_pool,
            tc.tile_pool(name="outsb", bufs=3) as out_pool,
            tc.tile_pool(name="ps", bufs=1, space="PSUM") as ps,
        ):
            # ---- input DMAs first: q weights + x chunk 0 gate the pipeline
            wq_sb = w_pool.tile([128, DIT, DC], bf16)
            nc.sync.dma_start(wq_sb[:], wq_d[:].rearrange("(t p) c -> p t c", p=128))
            xch0 = xin_pool.tile([128, DIT, 512], bf16, tag="xch", bufs=3)
            nc.sync.dma_start(
                xch0[:, 0 : DIT // 2, :],
                xT_d[0 : d // 2, 0:512].rearrange("(t p) c -> p t c", p=128),
            )
            nc.sync.dma_start(
                xch0[:, DIT // 2 :, :],
                xT_d[d // 2 :, 0:512].rearrange("(t p) c -> p t c", p=128),
            )
            wk_sb = w_pool.tile([128, DIT, DC], bf16)
            wv_sb = w_pool.tile([128, DIT, DC], bf16)
            nc.sync.dma_start(wk_sb[:], wk_d[:].rearrange("(t p) c -> p t c", p=128))
            nc.sync.dma_start(wv_sb[:], wv_d[:].rearrange("(t p) c -> p t c", p=128))
            wo_sb = w_pool.tile([DC, d], bf16)
            nc.sync.dma_start(wo_sb[:], wo_d[:])

            # ---- constants ----
            ident = const_pool.tile([128, 128], bf16)
            make_identity(nc, ident[:])
            # PE warm-up burst: ~3.5us of real matmul activity releases the
            # HAM clock throttle before the first projection arrives.
            warm_ps = ps.tile([128, 128], fp32, tag="gen", bufs=2,
                              padded_shape=[128, 512], name="gen_w")
            for _ in range(36):
                nc.tensor.matmul(warm_ps[:], ident[:], ident[:],
                                 start=True, stop=True)
            warm_sink = const_pool.tile([1, 128], fp32)
            nc.vector.tensor_copy(warm_sink[:], warm_ps[0:1, :])
            ones_row = const_pool.tile([1, 128], bf16)
            nc.gpsimd.memset(ones_row[:], 1.0)
            # Diagonal causal masks: mask[j][kl, ql] = 1 if ql >= kl + 128*j
            masks = const_pool.tile([128, 4, 512], bf16)
            nc.gpsimd.memset(masks[:], 1.0)
            for dd in range(4):
                nc.gpsimd.affine_select(
                    out=masks[:, dd, :],
                    in_=masks[:, dd, :],
                    compare_op=mybir.AluOpType.is_ge,
                    fill=0.0,
                    base=-128 * dd,
                    pattern=[[1, 512]],
                    channel_multiplier=-1,
                )

            # ---- persistent activations ----
            qt_sb = big_pool.tile([DC, n], bf16)  # Q^T (head dims on partitions)
            kt_sb = big_pool.tile([DC, n], bf16)  # K^T
            # V natural, augmented with ones columns at 64 (h0) and 144 (h1).
            # Per-head slices start at 32B-aligned offsets (0 and 160B) so the
            # DMA-xbar transposes can write them directly.
            v_aug = big_pool.tile([128, NT, 160], bf16)
            onescol = const_pool.tile([128, NT], fp32)
            nc.gpsimd.memset(onescol[:], 1.0)
            nc.vector.tensor_copy(v_aug[:, :, HD], onescol[:])
            nc.vector.tensor_copy(v_aug[:, :, 80 + HD], onescol[:])
            ctxn_sb = big_pool.tile([DC, n], bf16)  # normalized ctx^T

            # ---------- emission helpers ----------
            bg_queue = []  # deferred background closures (PE-filler work)

            def drain_bg(k):
                for _ in range(k):
                    if not bg_queue:
                        return
                    bg_queue.pop(0)()

            def emit_xch_dma(c):
                c0, c1 = c * 512, c * 512 + 512
                xch = xin_pool.tile([128, DIT, 512], bf16, tag="xch", bufs=3)
                nc.sync.dma_start(
                    xch[:], xT_d[:, c0:c1].rearrange("(t p) c -> p t c", p=128)
                )
                return xch

            def proj_units(c, xch, w_sb, evict):
                """Two 4-matmul halves of one projection (finer PE granularity)."""
                state = {}

                def first():
                    state["acc"] = ps.tile(
                        [128, 512], fp32, tag="gen", bufs=2, name="gen"
                    )
                    for dit in range(DIT // 2):
                        nc.tensor.matmul(
                            state["acc"][:], w_sb[:, dit, :], xch[:, dit, :],
                            start=(dit == 0), stop=False,
                        )

                def second():
                    for dit in range(DIT // 2, DIT):
                        nc.tensor.matmul(
                            state["acc"][:], w_sb[:, dit, :], xch[:, dit, :],
                            start=False, stop=(dit == DIT - 1),
                        )
                    evict(state["acc"])

                return [first, second]

            def qk_units(c, xch):
                qs = slice(c * 512, c * 512 + 512)

                def q_evict(acc):
                    nc.vector.tensor_copy(qt_sb[:, qs], acc[:])

                def k_evict(acc):
                    nc.vector.tensor_copy(kt_sb[:, qs], acc[:])

                return (proj_units(c, xch, wq_sb, q_evict)
                        + proj_units(c, xch, wk_sb, k_evict))

            def v_units(c, xch):
                vt_t = vt_pool.tile([DC, 512], bf16, tag="vt", bufs=3)

                def v_evict(acc):
                    nc.vector.tensor_copy(vt_t[:], acc[:])

                units = proj_units(c, xch, wv_sb, v_evict)

                def vtrans(j):
                    def unit():
                        ti = c * 4 + j
                        tp = ps.tile(
                            [128, 128], bf16, tag="gen", bufs=2,
                            padded_shape=[128, 512], name="gen_tp",
                        )
                        nc.tensor.transpose(
                            tp[:], vt_t[:, j * 128 : (j + 1) * 128], ident[:]
                        )
                        # one strided copy fills h0 cols 0:64 and h1 cols 80:144
                        nc.vector.tensor_copy(
                            v_aug[:, ti, 0:160].rearrange("p (g c) -> p g c", g=2)[
                                :, :, 0:HD
                            ],
                            tp[:].rearrange("p (g c) -> p g c", g=2),
                        )
                    return unit

                units += [vtrans(j) for j in range(4)]
                return units

            def score_exp(c, kt, sm_out):
                """Scores + exp for one step; returns the pm tile."""
                j = kt - 4 * c  # >= 0 on diagonal blocks
                qlo = 128 * j if j >= 0 else 0
                qs0 = c * 512
                kc = slice(kt * 128, kt * 128 + 128)
                nc.tensor.matmul(
                    sm_out[:, 0, qlo:512], kt_sb[0:HD, kc],
                    qt_sb[0:HD, qs0 + qlo : qs0 + 512],
                    start=True, stop=True, tile_position=(0, 0),
                )
                nc.tensor.matmul(
                    sm_out[:, 1, qlo:512], kt_sb[HD:DC, kc],
                    qt_sb[HD:DC, qs0 + qlo : qs0 + 512],
                    start=True, stop=True, tile_position=(64, 0),
                )
                pm = p_pool.tile([128, 2, 512], bf16, tag="p", bufs=4)
                nc.scalar.activation(
                    pm[:, :, qlo:512], sm_out[:, :, qlo:512], Exp, scale=SCALE
                )
                if j >= 0:
                    nc.vector.tensor_mul(
                        pm[:, 0, qlo:512], pm[:, 0, qlo:512], masks[:, j, qlo:512]
                    )
                    nc.vector.tensor_mul(
                        pm[:, 1, qlo:512], pm[:, 1, qlo:512], masks[:, j, qlo:512]
                    )
                return pm

            def pv_step(c, kt, nkt, ctx, pm):
                j = kt - 4 * c
                qlo = 128 * j if j >= 0 else 0
                nc.tensor.matmul(
                    ctx[:, 0, qlo:512], v_aug[:, kt, 0 : HD + 1], pm[:, 0, qlo:512],
                    start=(kt == 0), stop=(kt == nkt - 1),
                )
                nc.tensor.matmul(
                    ctx[:, 1, qlo:512], v_aug[:, kt, 80 : 80 + HD + 1],
                    pm[:, 1, qlo:512],
                    start=(kt == 0), stop=(kt == nkt - 1),
                )

            def epilogue_dve(c, ctx, last=False):
                """Immediately free the ctx PSUM banks: sums + raw eviction.

                The scalar engine is idle at chunk boundaries, so it takes the
                sums row and one ctx half in parallel with the DVE half —
                ctx's banks release ~2.5x sooner, unblocking the next chunk's
                first PV accumulation.
                """
                sums = ep_pool.tile([1, 2, 512], bf16, tag="sums", bufs=2)
                nc.vector.tensor_copy(sums[:], ctx[HD : HD + 1, :, :])
                craw = ep_pool.tile([128, 1024], bf16, tag="craw", bufs=2)
                nc.vector.tensor_copy(craw[0:HD, 0:512], ctx[0:HD, 0, :])
                if last:
                    nc.scalar.copy(craw[HD:DC, 512:1024], ctx[0:HD, 1, :])
                else:
                    nc.vector.tensor_copy(craw[HD:DC, 512:1024], ctx[0:HD, 1, :])
                return sums, craw

            def norm_units(c, sums, craw):
                """Normalize ctx^T for chunk c -> ctxn_sb (background units)."""
                qs = slice(c * 512, c * 512 + 512)
                rscb = ep_pool.tile([128, 1024], fp32, tag="rscb", bufs=2)

                def bcast(h):
                    def unit():
                        smb = ps.tile(
                            [128, 512], fp32, tag="gen", bufs=2, name="gen_b"
                        )
                        nc.tensor.matmul(
                            smb[:], ones_row[:], sums[:, h, :],
                            start=True, stop=True,
                        )
                        nc.vector.reciprocal_approx_fast(
                            rscb[:, h * 512 : h * 512 + 512], smb[:]
                        )
                    return unit

                def tnorm():
                    nc.vector.tensor_mul(
                        ctxn_sb[0:HD, qs], craw[0:HD, 0:512], rscb[0:HD, 0:512]
                    )
                    nc.vector.tensor_mul(
                        ctxn_sb[HD:DC, qs], craw[HD:DC, 512:1024],
                        rscb[HD:DC, 512:1024],
                    )

                return [bcast(0), bcast(1), tnorm]

            def outproj_units(c, last=False):
                """Out-projection for chunk c: 4 q-tiles x 2 d-halves."""
                units = []
                for jq in range(4):
                    jj = c * 4 + jq
                    gsl = slice(jj * 128, jj * 128 + 128)
                    o_sb = out_pool.tile([128, d], bf16, tag="o", bufs=3)

                    def half(h2, jj=jj, gsl=gsl, o_sb=o_sb):
                        def unit():
                            osl = slice(h2 * 512, h2 * 512 + 512)
                            op = ps.tile(
                                [128, 512], fp32, tag="gen", bufs=2, name="gen_o"
                            )
                            nc.tensor.matmul(
                                op[:], ctxn_sb[:, gsl], wo_sb[:, osl],
                                start=True, stop=True,
                            )
                            if last and h2 == 0:
                                nc.scalar.copy(o_sb[:, osl], op[:])
                            else:
                                nc.vector.tensor_copy(o_sb[:, osl], op[:])
                            if h2 == 1:
                                nc.sync.dma_start(out_d[gsl, :], o_sb[:])
                        return unit

                    units += [half(0), half(1)]
                return units

            # ---------- fused main loop ----------
            # prologue: chunk 0 q/k projections inline; v + transposes in bg.
            # QKV for chunk c+2 is enqueued during attention of chunk c
            # (pipeline depth 2) so the thin early chunks keep the PE dense.
            for u in qk_units(0, xch0):
                u()
            bg_queue.extend(v_units(0, xch0))
            xch1 = emit_xch_dma(1)
            bg_queue.extend(qk_units(1, xch1))
            bg_queue.extend(v_units(1, xch1))
            for c in range(NCH):
                if c + 2 < NCH:
                    xch = emit_xch_dma(c + 2)
                    bg_queue.extend(qk_units(c + 2, xch))
                    bg_queue.extend(v_units(c + 2, xch))
                nkt = 4 * (c + 1)
                ctx = ps.tile([HD + 1, 2, 512], fp32, tag="ctx", bufs=1, name="ctx")
                per_step = max(1, -(-len(bg_queue) // nkt))
                pm_prev = None
                for kt in range(nkt):
                    sm = ps.tile([128, 2, 512], fp32, tag="s", bufs=2, name="sm")
                    pm = score_exp(c, kt, sm)
                    if pm_prev is not None:
                        pv_step(c, kt - 1, nkt, ctx, pm_prev)
                    pm_prev = pm
                    drain_bg(per_step)
                pv_step(c, nkt - 1, nkt, ctx, pm_prev)
                drain_bg(len(bg_queue))
                last = c == NCH - 1
                sums, craw = epilogue_dve(c, ctx, last=last)
                bg_queue.extend(norm_units(c, sums, craw))
                bg_queue.extend(outproj_units(c, last=last))
            drain_bg(len(bg_queue))

    nc.compile()
    return nc


_NC_CACHE = {}


def _get_nc(n=SEQ):
    if n not in _NC_CACHE:
        _NC_CACHE[n] = build_bass(n)
    return _NC_CACHE[n]


def make_in_maps(x, W_q, W_k, W_v, W_o):
    import ml_dtypes

    bf16 = ml_dtypes.bfloat16
    n = x.shape[-2]
    xT = np.ascontiguousarray(
        np.asarray(x, dtype=np.float32).reshape(n, D).T
    ).astype(bf16)
    in_maps = []
    for c in range(N_CORES):
        s = slice(c * DC, (c + 1) * DC)
        in_maps.append(
            {
                "xT": xT,
                "wq": np.ascontiguousarray(np.asarray(W_q, np.float32)[:, s]).astype(bf16),
                "wk": np.ascontiguousarray(np.asarray(W_k, np.float32)[:, s]).astype(bf16),
                "wv": np.ascontiguousarray(np.asarray(W_v, np.float32)[:, s]).astype(bf16),
                "wo": np.ascontiguousarray(np.asarray(W_o, np.float32)[s, :]).astype(bf16),
            }
        )
    return in_maps


def kernel(x, W_q, W_k, W_v, W_o, b_o):
    from concourse import bass_utils

    x = np.asarray(x)
    b, n, _ = x.shape
    assert b == 1 and n == SEQ

    nc = _get_nc(n)
    in_maps = make_in_maps(x, W_q, W_k, W_v, W_o)
    res = bass_utils.run_bass_kernel_spmd(nc, in_maps, list(range(N_CORES)))
    acc = np.zeros((n, D), dtype=np.float64)
    for r in res.results:
        acc += np.asarray(r["out"], dtype=np.float64)
    acc += np.asarray(b_o, np.float64)[None, :]
    return acc.astype(np.float32).reshape(1, n, D)
